# revision 10
# baseline (speedup 1.0000x reference)
"""Trainium2 Bass kernel for nn_Aggregator (GNN message passing).

Strategy (8 NeuronCores, SPMD):
  - entity_agg rows sharded by head across cores (12544 rows/core); user_agg
    rows sharded by interact row (6272 rows/core).
  - Host sorts each core's edges by (tail_chunk, head_window); bucket sizes
    are equalized across cores so one program serves all 8 cores.
  - Device: dma_gather fetches entity rows for edges (int16 indices within a
    25088-row chunk); relation rows are applied via one-hot matmuls against
    the SBUF-resident relation table; segment-sum happens as one-hot matmuls
    accumulating in PSUM per 128-row output window, flushed into an
    SBUF-resident accumulator; a final pass divides by counts (entity) and
    applies the softmax-attention epilogue (user) before streaming to DRAM.
"""
import sys

sys.path.insert(0, "/opt/trn_rl_repo")

import numpy as np

import concourse.bass as bass
import concourse.bacc as bacc
import concourse.mybir as mybir
import concourse.tile as tile
from concourse.bass_utils import run_bass_kernel_spmd
from concourse.masks import make_identity

F32 = mybir.dt.float32
I16 = mybir.dt.int16

P = 128          # partitions / window size / block size
NIDX = 2048      # indices per dma_gather (16 blocks)
BPG = NIDX // P  # blocks per gather group
QUAD = 4         # blocks batched per DVE one-hot build
STRIP = 512      # metadata blocks per resident strip


class Phase:
    pass


# ---------------------------------------------------------------------------
# host-side schedule/metadata prep
# ---------------------------------------------------------------------------
def prep_phase(dst, src, payload, n_cores, rows_per_core, table_rows, chunk):
    """dst: segment ids [N] (output rows, sharded by core). src: gather rows.
    payload: per-edge f32 (edge type or interact value)."""
    ph = Phase()
    ph.rows_per_core = rows_per_core
    ph.n_win = rows_per_core // P
    ph.table_rows = table_rows
    ph.chunk = chunk
    n_win = ph.n_win
    n_chunk = ph.n_chunk = table_rows // chunk

    core = dst // rows_per_core
    win = (dst % rows_per_core) // P
    hl = (dst % rows_per_core) % P
    ck = src // chunk
    idx = src % chunk

    counts = np.zeros((n_cores, n_chunk, n_win), dtype=np.int64)
    np.add.at(counts, (core, ck, win), 1)
    eq = counts.max(axis=0)  # [n_chunk, n_win]
    for c in range(n_chunk):
        eq[c, n_win - 1] += (-eq[c].sum()) % P

    order = np.lexsort((idx, win, ck, core))
    s_core, s_ck, s_win = core[order], ck[order], win[order]
    s_idx, s_hl, s_pl = idx[order], hl[order], payload[order]

    bucket_off = np.zeros((n_chunk, n_win + 1), dtype=np.int64)
    bucket_off[:, 1:] = np.cumsum(eq, axis=1)
    chunk_len = bucket_off[:, -1]
    chunk_base = np.zeros(n_chunk + 1, dtype=np.int64)
    chunk_base[1:] = np.cumsum(chunk_len)
    total = int(chunk_base[-1])
    assert total % P == 0
    nb = ph.n_blocks = total // P
    ph.chunk_blocks = [int(chunk_len[c]) // P for c in range(n_chunk)]

    # per-edge equalized positions
    key = (s_core * n_chunk + s_ck) * n_win + s_win
    start = np.r_[0, np.flatnonzero(np.diff(key)) + 1]
    runlen = np.diff(np.r_[start, key.size])
    within = np.arange(key.size, dtype=np.int64) - np.repeat(start, runlen)
    pos = chunk_base[s_ck] + bucket_off[s_ck, s_win] + within

    # block schedule: per block, list of (w, local_base, start, stop) + flushes
    w_lo_blk = np.zeros(nb, dtype=np.int64)
    blocks = []
    blk0 = 0
    for c in range(n_chunk):
        cb = int(chunk_base[c])
        ends = cb + bucket_off[c, 1:]
        starts = cb + bucket_off[c, :-1]
        for k in range(ph.chunk_blocks[c]):
            s = cb + k * P
            e = s + P
            w_lo = int(np.searchsorted(ends, s, side="right"))
            w_hi = int(np.searchsorted(ends, e - 1, side="right"))
            assert w_hi <= w_lo + 1, f"block spans >2 buckets (c{c} b{k})"
            w_lo_blk[blk0 + k] = w_lo
            mms, flush = [], []
            for w in range(w_lo, min(w_hi, n_win - 1) + 1):
                if starts[w] == ends[w]:
                    continue
                mms.append((w, (w - w_lo) * P,
                            bool(starts[w] >= s), bool(ends[w] <= e)))
                if ends[w] <= e:
                    flush.append(w)
            blocks.append((mms, flush))
        blk0 += ph.chunk_blocks[c]
    ph.blocks = blocks

    # metadata arrays
    idx_full = np.zeros((n_cores, total), dtype=np.int16)
    hl_full = np.full((n_cores, total), -1.0, dtype=np.float32)
    pl_full = np.zeros((n_cores, total), dtype=np.float32)
    hl_adj = s_hl + P * (s_win - w_lo_blk[pos // P])
    assert (hl_adj >= 0).all() and (hl_adj < 2 * P).all()
    idx_full[s_core, pos] = s_idx.astype(np.int16)
    hl_full[s_core, pos] = hl_adj.astype(np.float32)
    pl_full[s_core, pos] = s_pl

    # gather groups (within chunks)
    groups = []
    blk0 = 0
    for c in range(n_chunk):
        k = 0
        while k < ph.chunk_blocks[c]:
            g = min(BPG, ph.chunk_blocks[c] - k)
            groups.append((c, blk0 + k, g))
            k += g
        blk0 += ph.chunk_blocks[c]
    ph.groups = groups

    ng = len(groups)
    idx_t = np.zeros((n_cores, ng, P, P), dtype=np.int16)
    for gi, (c, b0, gb) in enumerate(groups):
        seg = idx_full[:, b0 * P:(b0 + gb) * P]
        w = seg.reshape(n_cores, gb * 8, 16).transpose(0, 2, 1)
        idx_t[:, gi, :16, :gb * 8] = w
        idx_t[:, gi, 16:, :] = np.tile(idx_t[:, gi, :16, :], (1, 7, 1))
    ph.idx_t = idx_t
    ph.hl_t = np.ascontiguousarray(hl_full.reshape(n_cores, nb, P).transpose(0, 2, 1))
    ph.pl_t = np.ascontiguousarray(pl_full.reshape(n_cores, nb, P).transpose(0, 2, 1))
    return ph


# ---------------------------------------------------------------------------
# device program
# ---------------------------------------------------------------------------
def build_program(ph_e, ph_u, n_cores, upc, n_rel, n_fac):
    nc = bacc.Bacc("TRN2", target_bir_lowering=False, debug=False,
                   num_devices=n_cores)
    D = P

    table = nc.dram_tensor("table", [ph_e.table_rows, D], F32, kind="ExternalInput")
    e_idx = nc.dram_tensor("e_idx", [len(ph_e.groups), P, P], I16, kind="ExternalInput")
    e_hl = nc.dram_tensor("e_hl", [P, ph_e.n_blocks], F32, kind="ExternalInput")
    e_pl = nc.dram_tensor("e_pl", [P, ph_e.n_blocks], F32, kind="ExternalInput")
    e_recip = nc.dram_tensor("e_recip", [P, ph_e.n_win], F32, kind="ExternalInput")
    u_idx = nc.dram_tensor("u_idx", [len(ph_u.groups), P, P], I16, kind="ExternalInput")
    u_hl = nc.dram_tensor("u_hl", [P, ph_u.n_blocks], F32, kind="ExternalInput")
    u_pl = nc.dram_tensor("u_pl", [P, ph_u.n_blocks], F32, kind="ExternalInput")
    rel_emb = nc.dram_tensor("rel_emb", [4 * n_rel, D], F32, kind="ExternalInput")
    latentT = nc.dram_tensor("latentT", [D, n_fac], F32, kind="ExternalInput")
    disen = nc.dram_tensor("disen", [n_fac, n_rel], F32, kind="ExternalInput")
    user_emb = nc.dram_tensor("user_emb", [upc, D], F32, kind="ExternalInput")
    iota128x4 = nc.dram_tensor("iota128x4", [P, 4 * P], F32, kind="ExternalInput")
    iota32x4 = nc.dram_tensor("iota32x4", [P, P], F32, kind="ExternalInput")
    iota128p = nc.dram_tensor("iota128p", [P, P], F32, kind="ExternalInput")
    out_e = nc.dram_tensor("out_e", [ph_e.rows_per_core, D], F32, kind="ExternalOutput")
    out_u = nc.dram_tensor("out_u", [ph_u.rows_per_core, D], F32, kind="ExternalOutput")

    with tile.TileContext(nc) as tc:
        NS = 8
        gsems = [nc.alloc_semaphore(f"gsem{i}") for i in range(NS)]
        gcount = [0]

        def gw(k):
            return gsems[k % NS], 16 * (k // NS + 1)

        with tc.tile_pool(name="persist", bufs=1) as pp:
            acc_e = pp.tile([P, ph_e.rows_per_core], F32)
            acc_u = pp.tile([P, ph_u.rows_per_core], F32)
            nc.vector.memset(acc_e[:], 0.0)
            nc.vector.memset(acc_u[:], 0.0)
            io128x4 = pp.tile([P, 4 * P], F32)
            nc.sync.dma_start(io128x4[:], iota128x4[:])
            io32x4 = pp.tile([P, P], F32)
            nc.sync.dma_start(io32x4[:], iota32x4[:])
            io128p = pp.tile([P, P], F32)
            nc.sync.dma_start(io128p[:], iota128p[:])
            rel_sb = pp.tile([4 * n_rel, D], F32)
            nc.sync.dma_start(rel_sb[:], rel_emb[:])
            ident = pp.tile([P, P], F32)
            make_identity(nc, ident[:])

            def load_strips(hl_dram, pl_dram, n_blocks, pfx):
                tiles = {}
                for s0 in range(0, n_blocks, STRIP):
                    w = min(STRIP, n_blocks - s0)
                    hlt = pp.tile([P, w], F32, tag=f"{pfx}hl{s0}", name=f"{pfx}hl{s0}")
                    nc.sync.dma_start(hlt[:], hl_dram[:, s0:s0 + w])
                    plt = pp.tile([P, w], F32, tag=f"{pfx}pl{s0}", name=f"{pfx}pl{s0}")
                    nc.sync.dma_start(plt[:], pl_dram[:, s0:s0 + w])
                    tiles[s0] = (hlt, plt)
                return tiles

            def gather_phase(ph, idx_dram, strips, acc, is_entity,
                             mp, gp, wp, psp, prp, ptp):
                pfx = "e" if is_entity else "u"
                alive = {}
                for gi, (c, b0, gb) in enumerate(ph.groups):
                    gath = gp.tile([P, BPG, P], F32, tag="gath", name=f"g{pfx}{gi}")
                    idxt = mp.tile([P, P], I16, tag="idx", name=f"ix{pfx}{gi}")
                    nidx = gb * P
                    k = gcount[0]
                    gcount[0] += 1
                    # slots cycle with bufs=3: before reusing a slot, the
                    # gather that read/wrote it 3 groups ago must have landed
                    ld = nc.sync.dma_start(idxt[:], idx_dram[gi, :, :])
                    if k >= 3:
                        ld._wait_ge(*gw(k - 3))
                    ksem, kval = gw(k)
                    g_ins = nc.gpsimd.dma_gather(
                        gath[:, :gb, :],
                        table[c * ph.chunk:(c + 1) * ph.chunk, :],
                        idxt[:, :gb * 8], nidx, nidx, D,
                        single_packet=False,
                    ).then_inc(ksem, 16)
                    if k >= 3:
                        g_ins._wait_ge(*gw(k - 3))
                    gwait = gw(k)

                    q0 = 0
                    while q0 < gb:
                        babs = b0 + q0
                        cs = babs % STRIP
                        nq = min(QUAD, gb - q0, STRIP - cs)
                        hlt, plt = strips[(babs // STRIP) * STRIP]

                        oh = wp.tile([P, QUAD * P], F32, tag="oh", name=f"oh{pfx}{gi}_{q0}")
                        hl_ap = hlt[:, cs:cs + nq].unsqueeze(-1).to_broadcast([P, nq, P])
                        nc.vector.tensor_tensor(
                            out=oh[:, :nq * P].rearrange("p (a b) -> p a b", b=P),
                            in0=hl_ap,
                            in1=io128x4[:, :nq * P].rearrange("p (a b) -> p a b", b=P),
                            op=mybir.AluOpType.is_equal)

                        if is_entity:
                            ohr = wp.tile([P, QUAD * 32], F32, tag="ohr", name=f"ohr{gi}_{q0}")
                            pl_ap = plt[:, cs:cs + nq].unsqueeze(-1).to_broadcast([P, nq, 32])
                            nc.vector.tensor_tensor(
                                out=ohr[:, :nq * 32].rearrange("p (a b) -> p a b", b=32),
                                in0=pl_ap,
                                in1=io32x4[:, :nq * 32].rearrange("p (a b) -> p a b", b=32),
                                op=mybir.AluOpType.is_equal)
                            relg = prp.tile([P, QUAD * P], F32, tag="relg",
                                            name=f"rg{gi}_{q0}", space="PSUM")
                            for b in range(nq):
                                trp = ptp.tile([32, P], F32, tag="trp",
                                               name=f"trp{gi}_{q0}_{b}",
                                               space="PSUM")
                                nc.tensor.transpose(
                                    trp[:], ohr[:, b * 32:(b + 1) * 32], ident[:])
                                relT = wp.tile([32, P], F32, tag="relT",
                                               name=f"rt{gi}_{q0}_{b}")
                                nc.vector.tensor_copy(out=relT[:], in_=trp[:])
                                nc.tensor.matmul(
                                    relg[:, b * P:(b + 1) * P],
                                    lhsT=relT[:],
                                    rhs=rel_sb[:32, :],
                                    start=True, stop=True)
                            prod = wp.tile([P, QUAD * P], F32, tag="prod",
                                           name=f"pr{gi}_{q0}")
                            nc.vector.tensor_tensor(
                                out=prod[:, :nq * P].rearrange("p (a b) -> p a b", b=P),
                                in0=gath[:, q0:q0 + nq, :],
                                in1=relg[:, :nq * P].rearrange("p (a b) -> p a b", b=P),
                                op=mybir.AluOpType.mult)._wait_ge(*gwait)
                        else:
                            pl_ap = plt[:, cs:cs + nq].unsqueeze(-1).to_broadcast([P, nq, P])
                            nc.vector.tensor_tensor(
                                out=oh[:, :nq * P].rearrange("p (a b) -> p a b", b=P),
                                in0=oh[:, :nq * P].rearrange("p (a b) -> p a b", b=P),
                                in1=pl_ap, op=mybir.AluOpType.mult)

                        for bq in range(nq):
                            blk = babs + bq
                            mms, flush = ph.blocks[blk]
                            if is_entity:
                                rhs = prod[:, bq * P:(bq + 1) * P]
                            else:
                                rhs = gath[:, q0 + bq, :]
                            for (w, lb, mm_start, mm_stop) in mms:
                                if w not in alive:
                                    alive[w] = psp.tile([P, P], F32, tag="pw",
                                                        name=f"pw{pfx}{gi}_{w}",
                                                        space="PSUM")
                                pw = alive[w]
                                if lb == 0:
                                    lhsT = oh[:, bq * P:(bq + 1) * P]
                                else:
                                    oh2 = wp.tile([P, P], F32, tag="oh2",
                                                  name=f"oh2{pfx}{blk}")
                                    nc.vector.tensor_tensor(
                                        out=oh2[:],
                                        in0=hlt[:, cs + bq:cs + bq + 1].to_broadcast([P, P]),
                                        in1=io128p[:], op=mybir.AluOpType.is_equal)
                                    if not is_entity:
                                        nc.vector.tensor_tensor(
                                            out=oh2[:], in0=oh2[:],
                                            in1=plt[:, cs + bq:cs + bq + 1].to_broadcast([P, P]),
                                            op=mybir.AluOpType.mult)
                                    lhsT = oh2[:]
                                mm = nc.tensor.matmul(pw[:], lhsT=lhsT, rhs=rhs,
                                                      start=mm_start, stop=mm_stop)
                                if not is_entity:
                                    mm._wait_ge(*gwait)
                            for w in flush:
                                pw = alive.pop(w)
                                nc.vector.tensor_tensor(
                                    out=acc[:, w * P:(w + 1) * P],
                                    in0=acc[:, w * P:(w + 1) * P], in1=pw[:],
                                    op=mybir.AluOpType.add)
                        q0 += nq
                assert not alive, f"unflushed windows {sorted(alive)}"

            with (
                tc.tile_pool(name="meta", bufs=3) as mp,
                tc.tile_pool(name="gath", bufs=3) as gp,
                tc.tile_pool(name="work", bufs=3) as wp,
                tc.tile_pool(name="psum_w", bufs=2, space="PSUM") as psp,
                tc.tile_pool(name="psum_rel", bufs=2, space="PSUM") as prp,
                tc.tile_pool(name="psum_t", bufs=2, space="PSUM") as ptp,
            ):
                strips_e = load_strips(e_hl, e_pl, ph_e.n_blocks, "e")
                gather_phase(ph_e, e_idx, strips_e, acc_e, True,
                             mp, gp, wp, psp, prp, ptp)

                # entity epilogue: divide by counts, store
                rec = pp.tile([P, ph_e.n_win], F32)
                nc.sync.dma_start(rec[:], e_recip[:])
                for w in range(ph_e.n_win):
                    ot = wp.tile([P, P], F32, tag="oute", name=f"oute{w}")
                    nc.vector.tensor_scalar(
                        out=ot[:], in0=acc_e[:, w * P:(w + 1) * P],
                        scalar1=rec[:, w:w + 1], scalar2=None,
                        op0=mybir.AluOpType.mult)
                    nc.sync.dma_start(out_e[w * P:(w + 1) * P, :], ot[:])

                strips_u = load_strips(u_hl, u_pl, ph_u.n_blocks, "u")
                gather_phase(ph_u, u_idx, strips_u, acc_u, False,
                             mp, gp, wp, psp, prp, ptp)

            # ------------- user epilogue -----------------------------------
            with (
                tc.tile_pool(name="ework", bufs=3) as ew,
                tc.tile_pool(name="epsum", bufs=4, space="PSUM") as ep,
            ):
                latT = pp.tile([P, n_fac], F32)
                nc.sync.dma_start(latT[:], latentT[:])
                dis = pp.tile([n_fac, n_rel], F32)
                nc.sync.dma_start(dis[:], disen[:])
                dmax = pp.tile([n_fac, 1], F32)
                nc.vector.tensor_reduce(out=dmax[:], in_=dis[:],
                                        axis=mybir.AxisListType.X,
                                        op=mybir.AluOpType.max)
                dneg = pp.tile([n_fac, 1], F32)
                nc.vector.tensor_scalar(out=dneg[:], in0=dmax[:], scalar1=-1.0,
                                        scalar2=None, op0=mybir.AluOpType.mult)
                dexp = pp.tile([n_fac, n_rel], F32)
                dsum = pp.tile([n_fac, 1], F32)
                nc.scalar.activation(dexp[:], dis[:],
                                     mybir.ActivationFunctionType.Exp,
                                     bias=dneg[:], scale=1.0, accum_out=dsum[:])
                drec = pp.tile([n_fac, 1], F32)
                nc.vector.reciprocal(drec[:], dsum[:])
                dsm = pp.tile([n_fac, n_rel], F32)
                nc.vector.tensor_scalar(out=dsm[:], in0=dexp[:], scalar1=drec[:],
                                        scalar2=None, op0=mybir.AluOpType.mult)
                dsT_p = ep.tile([n_rel, n_fac], F32, tag="ep", name="dsTp",
                                space="PSUM")
                nc.tensor.transpose(dsT_p[:], dsm[:], ident[:n_fac, :n_fac])
                dsT = pp.tile([n_rel, n_fac], F32)
                nc.vector.tensor_copy(out=dsT[:], in_=dsT_p[:])
                dw_p = ep.tile([n_fac, P], F32, tag="ep", name="dwp", space="PSUM")
                nc.tensor.matmul(dw_p[:], lhsT=dsT[:], rhs=rel_sb[:n_rel, :],
                                 start=True, stop=True)
                dw5 = pp.tile([n_fac + 1, P], F32)
                nc.vector.memset(dw5[:], 1.0)
                nc.vector.tensor_copy(out=dw5[:n_fac, :], in_=dw_p[:])

                for w in range(ph_u.n_win):
                    ut = ew.tile([P, P], F32, tag="ut", name=f"ut{w}")
                    nc.sync.dma_start(ut[:], user_emb[w * P:(w + 1) * P, :])
                    utT_p = ep.tile([P, P], F32, tag="ep", name=f"utTp{w}",
                                    space="PSUM")
                    nc.tensor.transpose(utT_p[:], ut[:], ident[:])
                    utT = ew.tile([P, P], F32, tag="utT", name=f"utT{w}")
                    nc.vector.tensor_copy(out=utT[:], in_=utT_p[:])
                    sc_p = ep.tile([P, n_fac], F32, tag="ep", name=f"scp{w}",
                                   space="PSUM")
                    nc.tensor.matmul(sc_p[:], lhsT=utT[:], rhs=latT[:],
                                     start=True, stop=True)
                    smax = ew.tile([P, 1], F32, tag="smax", name=f"smax{w}")
                    nc.vector.tensor_reduce(out=smax[:], in_=sc_p[:],
                                            axis=mybir.AxisListType.X,
                                            op=mybir.AluOpType.max)
                    sneg = ew.tile([P, 1], F32, tag="sneg", name=f"sneg{w}")
                    nc.vector.tensor_scalar(out=sneg[:], in0=smax[:], scalar1=-1.0,
                                            scalar2=None, op0=mybir.AluOpType.mult)
                    s5 = ew.tile([P, n_fac + 1], F32, tag="s5", name=f"s5{w}")
                    ssum = ew.tile([P, 1], F32, tag="ssum", name=f"ssum{w}")
                    nc.scalar.activation(s5[:, :n_fac], sc_p[:],
                                         mybir.ActivationFunctionType.Exp,
                                         bias=sneg[:], scale=1.0, accum_out=ssum[:])
                    srec = ew.tile([P, 1], F32, tag="srec", name=f"srec{w}")
                    nc.vector.reciprocal(srec[:], ssum[:])
                    nc.vector.tensor_scalar(out=s5[:, :n_fac], in0=s5[:, :n_fac],
                                            scalar1=srec[:], scalar2=None,
                                            op0=mybir.AluOpType.mult)
                    nc.vector.memset(s5[:, n_fac:n_fac + 1], 1.0)
                    s5T_p = ep.tile([n_fac + 1, P], F32, tag="ep", name=f"s5Tp{w}",
                                    space="PSUM")
                    nc.tensor.transpose(s5T_p[:], s5[:], ident[:])
                    s5T = ew.tile([n_fac + 1, P], F32, tag="s5T", name=f"s5T{w}")
                    nc.vector.tensor_copy(out=s5T[:], in_=s5T_p[:])
                    mod_p = ep.tile([P, P], F32, tag="ep", name=f"modp{w}",
                                    space="PSUM")
                    nc.tensor.matmul(mod_p[:], lhsT=s5T[:], rhs=dw5[:],
                                     start=True, stop=True)
                    res = ew.tile([P, P], F32, tag="res", name=f"res{w}")
                    nc.vector.tensor_tensor(out=res[:], in0=mod_p[:],
                                            in1=acc_u[:, w * P:(w + 1) * P],
                                            op=mybir.AluOpType.mult)
                    nc.sync.dma_start(out_u[w * P:(w + 1) * P, :], res[:])

    nc.compile()
    return nc


# ---------------------------------------------------------------------------
# public entry
# ---------------------------------------------------------------------------
def run(entity_emb, user_emb, latent_emb, relation_emb, edge_index, edge_type,
        interact_rows, interact_cols, interact_vals, disen_weight_att,
        n_cores=8, sim=False):
    n_ent, D = entity_emb.shape
    n_usr = user_emb.shape[0]
    n_fac, n_rel = disen_weight_att.shape
    assert D == 128

    epc = -(-n_ent // (n_cores * P)) * P
    upc = -(-n_usr // (n_cores * P)) * P
    ent_pad = epc * n_cores
    chunk = 25088 if n_ent > 32000 else -(-n_ent // P) * P
    tab_pad = -(-n_ent // chunk) * chunk

    head = np.asarray(edge_index[0], dtype=np.int64)
    tail = np.asarray(edge_index[1], dtype=np.int64)
    et = np.asarray(edge_type, dtype=np.float32)
    irow = np.asarray(interact_rows, dtype=np.int64)
    icol = np.asarray(interact_cols, dtype=np.int64)
    ival = np.asarray(interact_vals, dtype=np.float32)

    ph_e = prep_phase(head, tail, et, n_cores, epc, tab_pad, chunk)
    ph_u = prep_phase(irow, icol, ival, n_cores, upc, tab_pad, chunk)

    cnt = np.bincount(head, minlength=ent_pad).astype(np.float32)
    recip = 1.0 / np.maximum(cnt, 1.0)
    recip_t = np.ascontiguousarray(
        recip.reshape(n_cores, epc // P, P).transpose(0, 2, 1))

    table = np.zeros((tab_pad, D), dtype=np.float32)
    table[:n_ent] = np.asarray(entity_emb, dtype=np.float32)
    usr_pad = upc * n_cores
    uemb = np.zeros((usr_pad, D), dtype=np.float32)
    uemb[:n_usr] = np.asarray(user_emb, dtype=np.float32)

    iota128x4 = np.ascontiguousarray(
        np.broadcast_to(np.tile(np.arange(P, dtype=np.float32), 4), (P, 4 * P)))
    iota32x4 = np.ascontiguousarray(
        np.broadcast_to(np.tile(np.arange(32, dtype=np.float32), 4), (P, P)))
    iota128p = np.ascontiguousarray(
        np.broadcast_to(np.arange(P, dtype=np.float32) + P, (P, P)))
    latT = np.ascontiguousarray(np.asarray(latent_emb, dtype=np.float32).T)

    nc = build_program(ph_e, ph_u, n_cores, upc, n_rel, n_fac)

    in_maps = []
    for c in range(n_cores):
        in_maps.append({
            "table": table,
            "e_idx": np.ascontiguousarray(ph_e.idx_t[c]),
            "e_hl": ph_e.hl_t[c], "e_pl": ph_e.pl_t[c],
            "e_recip": recip_t[c],
            "u_idx": np.ascontiguousarray(ph_u.idx_t[c]),
            "u_hl": ph_u.hl_t[c], "u_pl": ph_u.pl_t[c],
            "rel_emb": np.ascontiguousarray(np.tile(np.asarray(relation_emb, dtype=np.float32), (4, 1))),
            "latentT": latT,
            "disen": np.asarray(disen_weight_att, dtype=np.float32),
            "user_emb": np.ascontiguousarray(uemb[c * upc:(c + 1) * upc]),
            "iota128x4": iota128x4, "iota32x4": iota32x4, "iota128p": iota128p,
        })

    if sim:
        from concourse.bass_interp import CoreSim
        results = []
        for c in range(n_cores):
            s = CoreSim(nc)
            for k, v in in_maps[c].items():
                s.tensor(k)[:] = v
            s.simulate()
            results.append({"out_e": s.tensor("out_e").copy(),
                            "out_u": s.tensor("out_u").copy()})
    else:
        results = run_bass_kernel_spmd(nc, in_maps, list(range(n_cores))).results

    ent = np.concatenate([r["out_e"] for r in results], axis=0)[:n_ent]
    usr = np.concatenate([r["out_u"] for r in results], axis=0)[:n_usr]
    return ent, usr


def kernel(**inputs):
    return run(**inputs)


# revision 12
# speedup vs baseline: 6678.9918x; 6678.9918x over previous
"""Trainium2 Bass kernel for nn_Aggregator (GNN message passing).

Strategy (8 NeuronCores, SPMD):
  - entity_agg rows sharded by head across cores (12544 rows/core); user_agg
    rows sharded by interact row (6272 rows/core).
  - Host sorts each core's edges by (tail_chunk, head_window); bucket sizes
    are equalized across cores so one program serves all 8 cores.
  - Device: dma_gather fetches entity rows for edges (int16 indices within a
    25088-row chunk); relation rows are applied via one-hot matmuls against
    the SBUF-resident relation table; segment-sum happens as one-hot matmuls
    accumulating in PSUM per 128-row output window, flushed into an
    SBUF-resident accumulator; a final pass divides by counts (entity) and
    applies the softmax-attention epilogue (user) before streaming to DRAM.
"""
import sys

sys.path.insert(0, "/opt/trn_rl_repo")

import numpy as np

import concourse.bass as bass
import concourse.bacc as bacc
import concourse.mybir as mybir
import concourse.tile as tile
from concourse.bass_utils import run_bass_kernel_spmd
from concourse.masks import make_identity

F32 = mybir.dt.float32
I16 = mybir.dt.int16

LAST_N_GROUPS = [0]  # set by run(): gather groups per core (for timing models)

P = 128          # partitions / window size / block size
NIDX = 2048      # indices per dma_gather (16 blocks)
BPG = NIDX // P  # blocks per gather group
QUAD = 4         # blocks batched per DVE one-hot build
STRIP = 512      # metadata blocks per resident strip


class Phase:
    pass


# ---------------------------------------------------------------------------
# host-side schedule/metadata prep
# ---------------------------------------------------------------------------
def prep_phase(dst, src, payload, n_cores, rows_per_core, table_rows, chunk):
    """dst: segment ids [N] (output rows, sharded by core). src: gather rows.
    payload: per-edge f32 (edge type or interact value)."""
    ph = Phase()
    ph.rows_per_core = rows_per_core
    ph.n_win = rows_per_core // P
    ph.table_rows = table_rows
    ph.chunk = chunk
    n_win = ph.n_win
    n_chunk = ph.n_chunk = table_rows // chunk

    core = dst // rows_per_core
    win = (dst % rows_per_core) // P
    hl = (dst % rows_per_core) % P
    ck = src // chunk
    idx = src % chunk

    counts = np.zeros((n_cores, n_chunk, n_win), dtype=np.int64)
    np.add.at(counts, (core, ck, win), 1)
    eq = counts.max(axis=0)  # [n_chunk, n_win]
    for c in range(n_chunk):
        eq[c, n_win - 1] += (-eq[c].sum()) % P

    order = np.lexsort((idx, win, ck, core))
    s_core, s_ck, s_win = core[order], ck[order], win[order]
    s_idx, s_hl, s_pl = idx[order], hl[order], payload[order]

    bucket_off = np.zeros((n_chunk, n_win + 1), dtype=np.int64)
    bucket_off[:, 1:] = np.cumsum(eq, axis=1)
    chunk_len = bucket_off[:, -1]
    chunk_base = np.zeros(n_chunk + 1, dtype=np.int64)
    chunk_base[1:] = np.cumsum(chunk_len)
    total = int(chunk_base[-1])
    assert total % P == 0
    nb = ph.n_blocks = total // P
    ph.chunk_blocks = [int(chunk_len[c]) // P for c in range(n_chunk)]

    # per-edge equalized positions
    key = (s_core * n_chunk + s_ck) * n_win + s_win
    start = np.r_[0, np.flatnonzero(np.diff(key)) + 1]
    runlen = np.diff(np.r_[start, key.size])
    within = np.arange(key.size, dtype=np.int64) - np.repeat(start, runlen)
    pos = chunk_base[s_ck] + bucket_off[s_ck, s_win] + within

    # block schedule: per block, list of (w, local_base, start, stop) + flushes
    w_lo_blk = np.zeros(nb, dtype=np.int64)
    blocks = []
    blk0 = 0
    for c in range(n_chunk):
        cb = int(chunk_base[c])
        ends = cb + bucket_off[c, 1:]
        starts = cb + bucket_off[c, :-1]
        for k in range(ph.chunk_blocks[c]):
            s = cb + k * P
            e = s + P
            w_lo = int(np.searchsorted(ends, s, side="right"))
            w_hi = int(np.searchsorted(ends, e - 1, side="right"))
            assert w_hi <= w_lo + 1, f"block spans >2 buckets (c{c} b{k})"
            w_lo_blk[blk0 + k] = w_lo
            mms, flush = [], []
            for w in range(w_lo, min(w_hi, n_win - 1) + 1):
                if starts[w] == ends[w]:
                    continue
                mms.append((w, (w - w_lo) * P,
                            bool(starts[w] >= s), bool(ends[w] <= e)))
                if ends[w] <= e:
                    flush.append(w)
            blocks.append((mms, flush))
        blk0 += ph.chunk_blocks[c]
    ph.blocks = blocks

    # metadata arrays
    idx_full = np.zeros((n_cores, total), dtype=np.int16)
    hl_full = np.full((n_cores, total), -1.0, dtype=np.float32)
    pl_full = np.zeros((n_cores, total), dtype=np.float32)
    hl_adj = s_hl + P * (s_win - w_lo_blk[pos // P])
    assert (hl_adj >= 0).all() and (hl_adj < 2 * P).all()
    idx_full[s_core, pos] = s_idx.astype(np.int16)
    hl_full[s_core, pos] = hl_adj.astype(np.float32)
    pl_full[s_core, pos] = s_pl

    # gather groups (within chunks)
    groups = []
    blk0 = 0
    for c in range(n_chunk):
        k = 0
        while k < ph.chunk_blocks[c]:
            g = min(BPG, ph.chunk_blocks[c] - k)
            groups.append((c, blk0 + k, g))
            k += g
        blk0 += ph.chunk_blocks[c]
    ph.groups = groups

    ng = len(groups)
    idx_t = np.zeros((n_cores, ng, P, P), dtype=np.int16)
    for gi, (c, b0, gb) in enumerate(groups):
        seg = idx_full[:, b0 * P:(b0 + gb) * P]
        w = seg.reshape(n_cores, gb * 8, 16).transpose(0, 2, 1)
        idx_t[:, gi, :16, :gb * 8] = w
        idx_t[:, gi, 16:, :] = np.tile(idx_t[:, gi, :16, :], (1, 7, 1))
    ph.idx_t = idx_t
    ph.hl_t = np.ascontiguousarray(hl_full.reshape(n_cores, nb, P).transpose(0, 2, 1))
    ph.pl_t = np.ascontiguousarray(pl_full.reshape(n_cores, nb, P).transpose(0, 2, 1))
    return ph


# ---------------------------------------------------------------------------
# device program
# ---------------------------------------------------------------------------
def build_program(ph_e, ph_u, n_cores, upc, n_rel, n_fac, repeat_gathers=1):
    nc = bacc.Bacc("TRN2", target_bir_lowering=False, debug=False,
                   num_devices=n_cores)
    D = P

    table = nc.dram_tensor("table", [ph_e.table_rows, D], F32, kind="ExternalInput")
    e_idx = nc.dram_tensor("e_idx", [len(ph_e.groups), P, P], I16, kind="ExternalInput")
    e_hl = nc.dram_tensor("e_hl", [P, ph_e.n_blocks], F32, kind="ExternalInput")
    e_pl = nc.dram_tensor("e_pl", [P, ph_e.n_blocks], F32, kind="ExternalInput")
    e_recip = nc.dram_tensor("e_recip", [P, ph_e.n_win], F32, kind="ExternalInput")
    u_idx = nc.dram_tensor("u_idx", [len(ph_u.groups), P, P], I16, kind="ExternalInput")
    u_hl = nc.dram_tensor("u_hl", [P, ph_u.n_blocks], F32, kind="ExternalInput")
    u_pl = nc.dram_tensor("u_pl", [P, ph_u.n_blocks], F32, kind="ExternalInput")
    rel_emb = nc.dram_tensor("rel_emb", [4 * n_rel, D], F32, kind="ExternalInput")
    latentT = nc.dram_tensor("latentT", [D, n_fac], F32, kind="ExternalInput")
    disen = nc.dram_tensor("disen", [n_fac, n_rel], F32, kind="ExternalInput")
    user_emb = nc.dram_tensor("user_emb", [upc, D], F32, kind="ExternalInput")
    iota128x4 = nc.dram_tensor("iota128x4", [P, 4 * P], F32, kind="ExternalInput")
    iota32x4 = nc.dram_tensor("iota32x4", [P, P], F32, kind="ExternalInput")
    iota128p = nc.dram_tensor("iota128p", [P, P], F32, kind="ExternalInput")
    out_e = nc.dram_tensor("out_e", [ph_e.rows_per_core, D], F32, kind="ExternalOutput")
    out_u = nc.dram_tensor("out_u", [ph_u.rows_per_core, D], F32, kind="ExternalOutput")

    with tile.TileContext(nc) as tc:
        NS = 8
        gsems = [nc.alloc_semaphore(f"gsem{i}") for i in range(NS)]
        gcount = [0]

        def gw(k):
            return gsems[k % NS], 16 * (k // NS + 1) * repeat_gathers

        with tc.tile_pool(name="persist", bufs=1) as pp:
            acc_e = pp.tile([P, ph_e.rows_per_core], F32)
            acc_u = pp.tile([P, ph_u.rows_per_core], F32)
            nc.vector.memset(acc_e[:], 0.0)
            nc.vector.memset(acc_u[:], 0.0)
            io128x4 = pp.tile([P, 4 * P], F32)
            nc.sync.dma_start(io128x4[:], iota128x4[:])
            io32x4 = pp.tile([P, P], F32)
            nc.sync.dma_start(io32x4[:], iota32x4[:])
            io128p = pp.tile([P, P], F32)
            nc.sync.dma_start(io128p[:], iota128p[:])
            rel_sb = pp.tile([4 * n_rel, D], F32)
            nc.sync.dma_start(rel_sb[:], rel_emb[:])
            ident = pp.tile([P, P], F32)
            make_identity(nc, ident[:])

            def load_strips(hl_dram, pl_dram, n_blocks, pfx):
                tiles = {}
                for s0 in range(0, n_blocks, STRIP):
                    w = min(STRIP, n_blocks - s0)
                    hlt = pp.tile([P, w], F32, tag=f"{pfx}hl{s0}", name=f"{pfx}hl{s0}")
                    nc.sync.dma_start(hlt[:], hl_dram[:, s0:s0 + w])
                    plt = pp.tile([P, w], F32, tag=f"{pfx}pl{s0}", name=f"{pfx}pl{s0}")
                    nc.sync.dma_start(plt[:], pl_dram[:, s0:s0 + w])
                    tiles[s0] = (hlt, plt)
                return tiles

            def gather_phase(ph, idx_dram, strips, acc, is_entity,
                             mp, gp, wp, psp, prp, ptp):
                pfx = "e" if is_entity else "u"
                alive = {}
                for gi, (c, b0, gb) in enumerate(ph.groups):
                    gath = gp.tile([P, BPG, P], F32, tag="gath", name=f"g{pfx}{gi}")
                    idxt = mp.tile([P, P], I16, tag="idx", name=f"ix{pfx}{gi}")
                    nidx = gb * P
                    k = gcount[0]
                    gcount[0] += 1
                    # slots cycle with bufs=3: before reusing a slot, the
                    # gather that read/wrote it 3 groups ago must have landed
                    ld = nc.sync.dma_start(idxt[:], idx_dram[gi, :, :])
                    if k >= 3:
                        ld._wait_ge(*gw(k - 3))
                    ksem, kval = gw(k)
                    for _rep in range(repeat_gathers):
                        g_ins = nc.gpsimd.dma_gather(
                            gath[:, :gb, :],
                            table[c * ph.chunk:(c + 1) * ph.chunk, :],
                            idxt[:, :gb * 8], nidx, nidx, D,
                            single_packet=False,
                        ).then_inc(ksem, 16)
                        if k >= 3 and _rep == 0:
                            g_ins._wait_ge(*gw(k - 3))
                    gwait = gw(k)

                    q0 = 0
                    while q0 < gb:
                        babs = b0 + q0
                        cs = babs % STRIP
                        nq = min(QUAD, gb - q0, STRIP - cs)
                        hlt, plt = strips[(babs // STRIP) * STRIP]

                        oh = wp.tile([P, QUAD * P], F32, tag="oh", name=f"oh{pfx}{gi}_{q0}")
                        hl_ap = hlt[:, cs:cs + nq].unsqueeze(-1).to_broadcast([P, nq, P])
                        nc.vector.tensor_tensor(
                            out=oh[:, :nq * P].rearrange("p (a b) -> p a b", b=P),
                            in0=hl_ap,
                            in1=io128x4[:, :nq * P].rearrange("p (a b) -> p a b", b=P),
                            op=mybir.AluOpType.is_equal)

                        if is_entity:
                            ohr = wp.tile([P, QUAD * 32], F32, tag="ohr", name=f"ohr{gi}_{q0}")
                            pl_ap = plt[:, cs:cs + nq].unsqueeze(-1).to_broadcast([P, nq, 32])
                            nc.vector.tensor_tensor(
                                out=ohr[:, :nq * 32].rearrange("p (a b) -> p a b", b=32),
                                in0=pl_ap,
                                in1=io32x4[:, :nq * 32].rearrange("p (a b) -> p a b", b=32),
                                op=mybir.AluOpType.is_equal)
                            relg = prp.tile([P, QUAD * P], F32, tag="relg",
                                            name=f"rg{gi}_{q0}", space="PSUM")
                            for b in range(nq):
                                trp = ptp.tile([32, P], F32, tag="trp",
                                               name=f"trp{gi}_{q0}_{b}",
                                               space="PSUM")
                                nc.tensor.transpose(
                                    trp[:], ohr[:, b * 32:(b + 1) * 32], ident[:])
                                relT = wp.tile([32, P], F32, tag="relT",
                                               name=f"rt{gi}_{q0}_{b}")
                                nc.vector.tensor_copy(out=relT[:], in_=trp[:])
                                nc.tensor.matmul(
                                    relg[:, b * P:(b + 1) * P],
                                    lhsT=relT[:],
                                    rhs=rel_sb[:32, :],
                                    start=True, stop=True)
                            prod = wp.tile([P, QUAD * P], F32, tag="prod",
                                           name=f"pr{gi}_{q0}")
                            nc.vector.tensor_tensor(
                                out=prod[:, :nq * P].rearrange("p (a b) -> p a b", b=P),
                                in0=gath[:, q0:q0 + nq, :],
                                in1=relg[:, :nq * P].rearrange("p (a b) -> p a b", b=P),
                                op=mybir.AluOpType.mult)._wait_ge(*gwait)
                        else:
                            pl_ap = plt[:, cs:cs + nq].unsqueeze(-1).to_broadcast([P, nq, P])
                            nc.vector.tensor_tensor(
                                out=oh[:, :nq * P].rearrange("p (a b) -> p a b", b=P),
                                in0=oh[:, :nq * P].rearrange("p (a b) -> p a b", b=P),
                                in1=pl_ap, op=mybir.AluOpType.mult)

                        for bq in range(nq):
                            blk = babs + bq
                            mms, flush = ph.blocks[blk]
                            if is_entity:
                                rhs = prod[:, bq * P:(bq + 1) * P]
                            else:
                                rhs = gath[:, q0 + bq, :]
                            for (w, lb, mm_start, mm_stop) in mms:
                                if w not in alive:
                                    alive[w] = psp.tile([P, P], F32, tag="pw",
                                                        name=f"pw{pfx}{gi}_{w}",
                                                        space="PSUM")
                                pw = alive[w]
                                if lb == 0:
                                    lhsT = oh[:, bq * P:(bq + 1) * P]
                                else:
                                    oh2 = wp.tile([P, P], F32, tag="oh2",
                                                  name=f"oh2{pfx}{blk}")
                                    nc.vector.tensor_tensor(
                                        out=oh2[:],
                                        in0=hlt[:, cs + bq:cs + bq + 1].to_broadcast([P, P]),
                                        in1=io128p[:], op=mybir.AluOpType.is_equal)
                                    if not is_entity:
                                        nc.vector.tensor_tensor(
                                            out=oh2[:], in0=oh2[:],
                                            in1=plt[:, cs + bq:cs + bq + 1].to_broadcast([P, P]),
                                            op=mybir.AluOpType.mult)
                                    lhsT = oh2[:]
                                mm = nc.tensor.matmul(pw[:], lhsT=lhsT, rhs=rhs,
                                                      start=mm_start, stop=mm_stop)
                                if not is_entity:
                                    mm._wait_ge(*gwait)
                            for w in flush:
                                pw = alive.pop(w)
                                nc.vector.tensor_tensor(
                                    out=acc[:, w * P:(w + 1) * P],
                                    in0=acc[:, w * P:(w + 1) * P], in1=pw[:],
                                    op=mybir.AluOpType.add)
                        q0 += nq
                assert not alive, f"unflushed windows {sorted(alive)}"

            with (
                tc.tile_pool(name="meta", bufs=3) as mp,
                tc.tile_pool(name="gath", bufs=3) as gp,
                tc.tile_pool(name="work", bufs=3) as wp,
                tc.tile_pool(name="psum_w", bufs=2, space="PSUM") as psp,
                tc.tile_pool(name="psum_rel", bufs=2, space="PSUM") as prp,
                tc.tile_pool(name="psum_t", bufs=2, space="PSUM") as ptp,
            ):
                strips_e = load_strips(e_hl, e_pl, ph_e.n_blocks, "e")
                gather_phase(ph_e, e_idx, strips_e, acc_e, True,
                             mp, gp, wp, psp, prp, ptp)

                # entity epilogue: divide by counts, store
                rec = pp.tile([P, ph_e.n_win], F32)
                nc.sync.dma_start(rec[:], e_recip[:])
                for w in range(ph_e.n_win):
                    ot = wp.tile([P, P], F32, tag="oute", name=f"oute{w}")
                    nc.vector.tensor_scalar(
                        out=ot[:], in0=acc_e[:, w * P:(w + 1) * P],
                        scalar1=rec[:, w:w + 1], scalar2=None,
                        op0=mybir.AluOpType.mult)
                    nc.sync.dma_start(out_e[w * P:(w + 1) * P, :], ot[:])

                strips_u = load_strips(u_hl, u_pl, ph_u.n_blocks, "u")
                gather_phase(ph_u, u_idx, strips_u, acc_u, False,
                             mp, gp, wp, psp, prp, ptp)

            # ------------- user epilogue -----------------------------------
            with (
                tc.tile_pool(name="ework", bufs=3) as ew,
                tc.tile_pool(name="epsum", bufs=4, space="PSUM") as ep,
            ):
                latT = pp.tile([P, n_fac], F32)
                nc.sync.dma_start(latT[:], latentT[:])
                dis = pp.tile([n_fac, n_rel], F32)
                nc.sync.dma_start(dis[:], disen[:])
                dmax = pp.tile([n_fac, 1], F32)
                nc.vector.tensor_reduce(out=dmax[:], in_=dis[:],
                                        axis=mybir.AxisListType.X,
                                        op=mybir.AluOpType.max)
                dneg = pp.tile([n_fac, 1], F32)
                nc.vector.tensor_scalar(out=dneg[:], in0=dmax[:], scalar1=-1.0,
                                        scalar2=None, op0=mybir.AluOpType.mult)
                dexp = pp.tile([n_fac, n_rel], F32)
                dsum = pp.tile([n_fac, 1], F32)
                nc.scalar.activation(dexp[:], dis[:],
                                     mybir.ActivationFunctionType.Exp,
                                     bias=dneg[:], scale=1.0, accum_out=dsum[:])
                drec = pp.tile([n_fac, 1], F32)
                nc.vector.reciprocal(drec[:], dsum[:])
                dsm = pp.tile([n_fac, n_rel], F32)
                nc.vector.tensor_scalar(out=dsm[:], in0=dexp[:], scalar1=drec[:],
                                        scalar2=None, op0=mybir.AluOpType.mult)
                dsT_p = ep.tile([n_rel, n_fac], F32, tag="ep", name="dsTp",
                                space="PSUM")
                nc.tensor.transpose(dsT_p[:], dsm[:], ident[:n_fac, :n_fac])
                dsT = pp.tile([n_rel, n_fac], F32)
                nc.vector.tensor_copy(out=dsT[:], in_=dsT_p[:])
                dw_p = ep.tile([n_fac, P], F32, tag="ep", name="dwp", space="PSUM")
                nc.tensor.matmul(dw_p[:], lhsT=dsT[:], rhs=rel_sb[:n_rel, :],
                                 start=True, stop=True)
                dw5 = pp.tile([n_fac + 1, P], F32)
                nc.vector.memset(dw5[:], 1.0)
                nc.vector.tensor_copy(out=dw5[:n_fac, :], in_=dw_p[:])

                for w in range(ph_u.n_win):
                    ut = ew.tile([P, P], F32, tag="ut", name=f"ut{w}")
                    nc.sync.dma_start(ut[:], user_emb[w * P:(w + 1) * P, :])
                    utT_p = ep.tile([P, P], F32, tag="ep", name=f"utTp{w}",
                                    space="PSUM")
                    nc.tensor.transpose(utT_p[:], ut[:], ident[:])
                    utT = ew.tile([P, P], F32, tag="utT", name=f"utT{w}")
                    nc.vector.tensor_copy(out=utT[:], in_=utT_p[:])
                    sc_p = ep.tile([P, n_fac], F32, tag="ep", name=f"scp{w}",
                                   space="PSUM")
                    nc.tensor.matmul(sc_p[:], lhsT=utT[:], rhs=latT[:],
                                     start=True, stop=True)
                    smax = ew.tile([P, 1], F32, tag="smax", name=f"smax{w}")
                    nc.vector.tensor_reduce(out=smax[:], in_=sc_p[:],
                                            axis=mybir.AxisListType.X,
                                            op=mybir.AluOpType.max)
                    sneg = ew.tile([P, 1], F32, tag="sneg", name=f"sneg{w}")
                    nc.vector.tensor_scalar(out=sneg[:], in0=smax[:], scalar1=-1.0,
                                            scalar2=None, op0=mybir.AluOpType.mult)
                    s5 = ew.tile([P, n_fac + 1], F32, tag="s5", name=f"s5{w}")
                    ssum = ew.tile([P, 1], F32, tag="ssum", name=f"ssum{w}")
                    nc.scalar.activation(s5[:, :n_fac], sc_p[:],
                                         mybir.ActivationFunctionType.Exp,
                                         bias=sneg[:], scale=1.0, accum_out=ssum[:])
                    srec = ew.tile([P, 1], F32, tag="srec", name=f"srec{w}")
                    nc.vector.reciprocal(srec[:], ssum[:])
                    nc.vector.tensor_scalar(out=s5[:, :n_fac], in0=s5[:, :n_fac],
                                            scalar1=srec[:], scalar2=None,
                                            op0=mybir.AluOpType.mult)
                    nc.vector.memset(s5[:, n_fac:n_fac + 1], 1.0)
                    s5T_p = ep.tile([n_fac + 1, P], F32, tag="ep", name=f"s5Tp{w}",
                                    space="PSUM")
                    nc.tensor.transpose(s5T_p[:], s5[:], ident[:])
                    s5T = ew.tile([n_fac + 1, P], F32, tag="s5T", name=f"s5T{w}")
                    nc.vector.tensor_copy(out=s5T[:], in_=s5T_p[:])
                    mod_p = ep.tile([P, P], F32, tag="ep", name=f"modp{w}",
                                    space="PSUM")
                    nc.tensor.matmul(mod_p[:], lhsT=s5T[:], rhs=dw5[:],
                                     start=True, stop=True)
                    res = ew.tile([P, P], F32, tag="res", name=f"res{w}")
                    nc.vector.tensor_tensor(out=res[:], in0=mod_p[:],
                                            in1=acc_u[:, w * P:(w + 1) * P],
                                            op=mybir.AluOpType.mult)
                    nc.sync.dma_start(out_u[w * P:(w + 1) * P, :], res[:])

    nc.compile()
    return nc


# ---------------------------------------------------------------------------
# public entry
# ---------------------------------------------------------------------------
def run(entity_emb, user_emb, latent_emb, relation_emb, edge_index, edge_type,
        interact_rows, interact_cols, interact_vals, disen_weight_att,
        n_cores=8, sim=False, repeat_gathers=1, verbose=False):
    import time as _time
    _t0 = _time.time()
    n_ent, D = entity_emb.shape
    n_usr = user_emb.shape[0]
    n_fac, n_rel = disen_weight_att.shape
    assert D == 128

    epc = -(-n_ent // (n_cores * P)) * P
    upc = -(-n_usr // (n_cores * P)) * P
    ent_pad = epc * n_cores
    chunk = 25088 if n_ent > 32000 else -(-n_ent // P) * P
    tab_pad = -(-n_ent // chunk) * chunk

    head = np.asarray(edge_index[0], dtype=np.int64)
    tail = np.asarray(edge_index[1], dtype=np.int64)
    et = np.asarray(edge_type, dtype=np.float32)
    irow = np.asarray(interact_rows, dtype=np.int64)
    icol = np.asarray(interact_cols, dtype=np.int64)
    ival = np.asarray(interact_vals, dtype=np.float32)

    ph_e = prep_phase(head, tail, et, n_cores, epc, tab_pad, chunk)
    ph_u = prep_phase(irow, icol, ival, n_cores, upc, tab_pad, chunk)

    cnt = np.bincount(head, minlength=ent_pad).astype(np.float32)
    recip = 1.0 / np.maximum(cnt, 1.0)
    recip_t = np.ascontiguousarray(
        recip.reshape(n_cores, epc // P, P).transpose(0, 2, 1))

    table = np.zeros((tab_pad, D), dtype=np.float32)
    table[:n_ent] = np.asarray(entity_emb, dtype=np.float32)
    usr_pad = upc * n_cores
    uemb = np.zeros((usr_pad, D), dtype=np.float32)
    uemb[:n_usr] = np.asarray(user_emb, dtype=np.float32)

    iota128x4 = np.ascontiguousarray(
        np.broadcast_to(np.tile(np.arange(P, dtype=np.float32), 4), (P, 4 * P)))
    iota32x4 = np.ascontiguousarray(
        np.broadcast_to(np.tile(np.arange(32, dtype=np.float32), 4), (P, P)))
    iota128p = np.ascontiguousarray(
        np.broadcast_to(np.arange(P, dtype=np.float32) + P, (P, P)))
    latT = np.ascontiguousarray(np.asarray(latent_emb, dtype=np.float32).T)

    LAST_N_GROUPS[0] = len(ph_e.groups) + len(ph_u.groups)
    if verbose:
        print(f"[kernel] host prep: {_time.time()-_t0:.1f}s  "
              f"groups e={len(ph_e.groups)} u={len(ph_u.groups)} "
              f"blocks e={ph_e.n_blocks} u={ph_u.n_blocks}")
    _t1 = _time.time()
    nc = build_program(ph_e, ph_u, n_cores, upc, n_rel, n_fac,
                       repeat_gathers=repeat_gathers)
    if verbose:
        print(f"[kernel] build+compile: {_time.time()-_t1:.1f}s")
    _t2 = _time.time()

    in_maps = []
    for c in range(n_cores):
        in_maps.append({
            "table": table,
            "e_idx": np.ascontiguousarray(ph_e.idx_t[c]),
            "e_hl": ph_e.hl_t[c], "e_pl": ph_e.pl_t[c],
            "e_recip": recip_t[c],
            "u_idx": np.ascontiguousarray(ph_u.idx_t[c]),
            "u_hl": ph_u.hl_t[c], "u_pl": ph_u.pl_t[c],
            "rel_emb": np.ascontiguousarray(np.tile(np.asarray(relation_emb, dtype=np.float32), (4, 1))),
            "latentT": latT,
            "disen": np.asarray(disen_weight_att, dtype=np.float32),
            "user_emb": np.ascontiguousarray(uemb[c * upc:(c + 1) * upc]),
            "iota128x4": iota128x4, "iota32x4": iota32x4, "iota128p": iota128p,
        })

    if verbose:
        print(f"[kernel] in_maps built: {_time.time()-_t2:.1f}s")
    _t3 = _time.time()
    if sim:
        from concourse.bass_interp import CoreSim
        results = []
        for c in range(n_cores):
            s = CoreSim(nc)
            for k, v in in_maps[c].items():
                s.tensor(k)[:] = v
            s.simulate()
            results.append({"out_e": s.tensor("out_e").copy(),
                            "out_u": s.tensor("out_u").copy()})
    else:
        results = run_bass_kernel_spmd(nc, in_maps, list(range(n_cores))).results

    if verbose:
        print(f"[kernel] spmd run: {_time.time()-_t3:.1f}s")
    ent = np.concatenate([r["out_e"] for r in results], axis=0)[:n_ent]
    usr = np.concatenate([r["out_u"] for r in results], axis=0)[:n_usr]
    return ent, usr


def kernel(**inputs):
    return run(**inputs)
